# revision 1
# baseline (speedup 1.0000x reference)
"""GAT spatial kernel for trn2 (nn_GATSpatial_36112085025002).

Strategy v2 (engine-rebalanced)
-------------------------------
Data-parallel over B=8 across the 8 NeuronCores; each core runs the full
2-layer GAT for one batch element.

Per-core algorithm (attention math in transposed layout sT[m(keys), q]):
  - projections in float32r (tf32-like, full matmul rate)
  - scores via K=65 augmented contraction: rows 0-63 = hT (fp16), row 64 is
    (k-side) ones / (q-side) -||h_q||^2.  The diagonal-score shift makes
    exp(s - r_q^2) overflow-free and any (per-q uniform) shift error cancels
    exactly between numerator and denominator.
  - ACT engine does ONLY the exp (PSUM f32 -> SBUF bf16); every other
    PSUM drain / elementwise op lives on DVE or GPSIMD.
  - mask applied after exp as a bf16 multiply on DVE (softmax shift
    invariance makes this exact w.r.t. the reference).
  - P@V with stationary H_aug [m,65] (fp16, col 64 = ones -> denominators
    come out as row 64 of the output for free).
  - leaky_relu is positively homogeneous, so it is FUSED into the PSUM
    drain (scalar_tensor_tensor (x*0.2) max x) BEFORE dividing by the
    softmax denominator; the normalize multiply runs on GPSIMD.
  - r_q^2 built by a ones-vector matmul over squared hT (no transposes of
    r2 needed); H_aug built by PE transposes batched 4-at-a-time with one
    strided DVE copy each.
  - aug build for head h+1 is software-pipelined INTO head h's attention
    mc-loop so the PE never idles long enough to re-throttle.
  - epilogue LN per 128-row block in transposed layout; output staged and
    written with ONE DMA per 1024-row block.
"""
import sys

sys.path.insert(0, '/opt/trn_rl_repo')

import numpy as np
import ml_dtypes

import concourse.bass as bass
import concourse.tile as tile
import concourse.mybir as mybir
from concourse.masks import make_identity

F32 = mybir.dt.float32
F32R = mybir.dt.float32r
F16 = mybir.dt.float16
BF16 = mybir.dt.bfloat16
AF = mybir.ActivationFunctionType
ALU = mybir.AluOpType
AX = mybir.AxisListType

N_CORES = 8
LN_EPS = 1e-5

# ---------------------------------------------------------------------------
# walrus workaround: this compiler build rejects >1 sync-wait per instruction.
# Split extra waits into standalone EventSemaphore instructions.
# ---------------------------------------------------------------------------
_orig_commit = tile.TileContext._commit_and_lower


def _patched_commit(self, inst, *args, **kwargs):
    si = getattr(inst, "sync_info", None)
    waits = list(si.on_wait) if si is not None and si.on_wait else []
    if len(waits) > 1:
        for w in waits[:-1]:
            ev = mybir.InstEventSemaphore(
                name=self.nc.get_next_instruction_name(),
                engine=inst.engine,
                ins=[],
                outs=[],
                sync_info=mybir.SyncInfo(on_wait=[w], on_update=[]),
            )
            _orig_commit(self, ev, *args, **kwargs)
        si.on_wait = [waits[-1]]
        inst.sync_info = si
    return _orig_commit(self, inst, *args, **kwargs)


def _patched_drain_and_barrier(self, tick_clock, wait_clock):
    from concourse.tile import ScopedClock

    nc = self.nc
    dummy = mybir.InstDrain(
        name="tail-drain-waits", ins=[], outs=[], bass_is_fusable=False
    )
    dummy.engine = nc.sync.engine
    wait_clock.add_sem_waits(dummy, ScopedClock({None: tick_clock.global_clock}))
    waits = list(dummy.sync_info.on_wait) if dummy.sync_info else []
    for w in waits:
        ev = mybir.InstEventSemaphore(
            name=nc.get_next_instruction_name(),
            engine=nc.sync.engine,
            ins=[],
            outs=[],
            sync_info=mybir.SyncInfo(on_wait=[w], on_update=[]),
        )
        nc.sync.add_instruction(ev)
    nc.sync.drain()

    nc.all_engine_barrier()
    assert self.sems is not None
    popped = nc._tile_sem_poison_stack.pop()
    assert popped is self._sem_poison
    nc.clear_and_free_semaphores(list(self.sems.allocated().values()))
    nc.all_engine_barrier()


if getattr(tile.TileContext, "_wait_split_patched", False) is False:
    tile.TileContext._commit_and_lower = _patched_commit
    tile.TileContext._drain_and_barrier = _patched_drain_and_barrier
    tile.TileContext._wait_split_patched = True


# ---------------------------------------------------------------------------
# Kernel builder
# ---------------------------------------------------------------------------
def build_gat(N=2048, C=64, H=4, D=64,
              use_bh=False, use_bo=False, use_gamma=False, use_beta=False):
    assert N % 512 == 0
    NT = N // 128                     # key chunks
    QB = min(1024, N)                 # q block
    NQB = N // QB
    NP = N // 512                     # 512-wide column parts
    HD = H * D

    nc = bass.Bass(trn_type="TRN2")
    xt_d = nc.dram_tensor("xt", [C, N], F32R, kind="ExternalInput")
    maskt_d = nc.dram_tensor("maskt", [N, N], BF16, kind="ExternalInput")
    wht_d = nc.dram_tensor("wht", [C, H * D], F32R, kind="ExternalInput")
    wot_d = nc.dram_tensor("wot", [128, (HD // 128) * D], F32R, kind="ExternalInput")
    bh_d = nc.dram_tensor("bh", [128, HD // 128], F32, kind="ExternalInput") if use_bh else None
    bo_d = nc.dram_tensor("bo", [D], F32, kind="ExternalInput") if use_bo else None
    gamma_d = nc.dram_tensor("gamma", [D], F32, kind="ExternalInput") if use_gamma else None
    beta_d = nc.dram_tensor("beta", [D], F32, kind="ExternalInput") if use_beta else None
    out_d = nc.dram_tensor("out", [N, D], F32, kind="ExternalOutput")

    with tile.TileContext(nc) as tc:
        import contextlib
        ctx = contextlib.ExitStack()
        with ctx:
            const = ctx.enter_context(tc.tile_pool(name="const", bufs=1))
            aug = ctx.enter_context(tc.tile_pool(name="aug", bufs=2))
            rowp = ctx.enter_context(tc.tile_pool(name="rowp", bufs=2))
            small = ctx.enter_context(tc.tile_pool(name="small", bufs=4))
            ppool = ctx.enter_context(tc.tile_pool(name="ppool", bufs=4))
            stage = ctx.enter_context(tc.tile_pool(name="stage", bufs=2))
            ps_sc = ctx.enter_context(tc.tile_pool(name="ps_sc", bufs=2, space="PSUM"))
            ps_ot = ctx.enter_context(tc.tile_pool(name="ps_ot", bufs=2, space="PSUM"))
            drb = ctx.enter_context(tc.tile_pool(name="drb", bufs=2, space="DRAM"))

            # ---- constants ----------------------------------------------------
            idf32 = const.tile([128, 128], F32, name="idf32")
            make_identity(nc, idf32)
            idf16 = const.tile([128, 128], F16, name="idf16")
            nc.vector.tensor_copy(idf16, idf32)
            idbf16 = const.tile([128, 128], BF16, name="idbf16")
            nc.vector.tensor_copy(idbf16, idf32)
            ones_negT = const.tile([64, 1], F16, name="ones_negT")
            nc.vector.memset(ones_negT, -1.0)

            # xT, weights: DMA straight into float32r tiles (same bit layout)
            xT = const.tile([C, N], F32R, name="xT")
            nc.sync.dma_start(xT, xt_d[:, :])
            whT_sb = const.tile([C, H * D], F32R, name="whT_sb")
            nc.sync.dma_start(whT_sb, wht_d[:, :])
            woT_sb = const.tile([128, 2 * D], F32R, name="woT_sb")
            nc.sync.dma_start(woT_sb, wot_d[:, :])

            # mask resident in SBUF: [128, NT*N] bf16, chunk mc at cols [mc*N, (mc+1)*N)
            mask_sb = const.tile([128, NT * N], BF16, name="mask_sb")
            for mc in range(NT):
                nc.sync.dma_start(mask_sb[:, mc * N:(mc + 1) * N],
                                  maskt_d[mc * 128:(mc + 1) * 128, :])

            bh_cols = None
            if use_bh:
                bh_cols = const.tile([128, 2], F32, name="bh_cols")
                nc.sync.dma_start(bh_cols, bh_d[:, :])
            bo_row = gamma_row = beta_row = None
            if use_bo:
                bo_row = const.tile([128, D], F32, name="bo_row")
                nc.sync.dma_start(bo_row, bo_d.to_broadcast([128, D]))
            if use_gamma:
                gamma_row = const.tile([128, D], F32, name="gamma_row")
                nc.sync.dma_start(gamma_row, gamma_d.to_broadcast([128, D]))
            if use_beta:
                beta_row = const.tile([128, D], F32, name="beta_row")
                nc.sync.dma_start(beta_row, beta_d.to_broadcast([128, D]))

            zT = [const.tile([128, N], F32R, name=f"zT{t}") for t in range(HD // 128)]

            # ---- aug build (returns named closures for injection) -------------
            def make_aug(proj_cb, tag):
                aug_q = aug.tile([65, N], F16, name=f"aq_{tag}", tag="aug_q")
                aug_k = aug.tile([65, N], F16, name=f"ak_{tag}", tag="aug_k")
                H_aug = aug.tile([128, NT * 65], F16, name=f"ha_{tag}", tag="H_aug")
                sq = aug.tile([64, N], F16, name=f"sq_{tag}", tag="sq")

                def s_ones():
                    nc.gpsimd.memset(aug_k[64:65, :], 1.0)
                    ones_ap = bass.AP(
                        tensor=H_aug.tensor, offset=H_aug.offset + 64,
                        ap=[H_aug.ap[0], [65, NT]])
                    nc.vector.memset(ones_ap, 1.0)

                def s_proj(p):
                    def f():
                        j0 = p * 512
                        ps = ps_sc.tile([64, 512], F32, name=f"prj_{tag}_{p}",
                                        tag="sc")
                        proj_cb(p, ps)
                        nc.vector.tensor_copy(aug_k[0:64, j0:j0 + 512], ps)
                    return f

                def s_gq(p):
                    def f():
                        j0 = p * 512
                        nc.gpsimd.tensor_copy(aug_q[0:64, j0:j0 + 512],
                                              aug_k[0:64, j0:j0 + 512])
                        nc.gpsimd.tensor_tensor(sq[:, j0:j0 + 512],
                                                aug_k[0:64, j0:j0 + 512],
                                                aug_k[0:64, j0:j0 + 512],
                                                op=ALU.mult)
                    return f

                def s_tr(g):
                    def f():
                        tpg = ps_sc.tile([128, 256], F16, name=f"tpg_{tag}_{g}",
                                         tag="sc")
                        for i in range(4):
                            mc = g * 4 + i
                            nc.tensor.transpose(
                                tpg[:, i * 64:(i + 1) * 64],
                                aug_k[0:64, mc * 128:(mc + 1) * 128],
                                idf16[:64, :64])
                        dst = bass.AP(
                            tensor=H_aug.tensor,
                            offset=H_aug.offset + g * 4 * 65,
                            ap=[H_aug.ap[0], [65, 4], [1, 64]])
                        srcv = bass.AP(
                            tensor=tpg.tensor, offset=tpg.offset,
                            ap=[tpg.ap[0], [64, 4], [1, 64]])
                        nc.vector.tensor_copy(dst, srcv)
                    return f

                def s_r2(p):
                    def f():
                        j0 = p * 512
                        r2 = ps_sc.tile([1, 512], F32, name=f"r2_{tag}_{p}",
                                        tag="sc")
                        nc.tensor.matmul(r2, ones_negT, sq[:, j0:j0 + 512],
                                         start=True, stop=True)
                        nc.vector.tensor_copy(aug_q[64:65, j0:j0 + 512], r2)
                    return f

                stages = [s_ones,
                          s_proj(0), s_proj(1), s_gq(0), s_proj(2), s_gq(1),
                          s_proj(3), s_gq(2), s_tr(0), s_gq(3), s_tr(1),
                          s_tr(2), s_tr(3),
                          s_r2(0), s_r2(1), s_r2(2), s_r2(3)]
                return aug_q, aug_k, H_aug, stages

            # ---- attention core ----------------------------------------------
            # Scores get the additive log-mask (-60 for masked entries) folded
            # in on the PE via an identity-stationary matmul, so exp output is
            # already masked and feeds P@V directly.  The P@V for chunk mc-1
            # is emitted AFTER the score matmuls for chunk mc so the in-order
            # PE never waits on the exp of the chunk it just produced; the
            # extra PE work keeps the PE saturated, which holds the HAM clock
            # gate at the full 2.4 GHz.
            def attention(aug_q, aug_k, H_aug, out_cb, tag, inject=None):
                inject = inject or {}
                for qb in range(NQB):
                    ot_ps = ps_ot.tile([65, QB], F32, name=f"ot_{tag}_{qb}",
                                       tag="ot")
                    pms = {}

                    def pv(mc):
                        pm = pms.pop(mc)
                        for nb in range(QB // 512):
                            nc.tensor.matmul(
                                ot_ps[:, nb * 512:(nb + 1) * 512],
                                H_aug[:, mc * 65:mc * 65 + 65],
                                pm[:, nb * 512:(nb + 1) * 512],
                                start=(mc == 0), stop=(mc == NT - 1))

                    for mc in range(NT):
                        sc = ps_sc.tile([128, QB], F32,
                                        name=f"sc_{tag}_{qb}_{mc}", tag="sc")
                        for nb in range(QB // 512):
                            cols = slice(qb * QB + nb * 512,
                                         qb * QB + (nb + 1) * 512)
                            nc.tensor.matmul(
                                sc[:, nb * 512:(nb + 1) * 512],
                                aug_k[:, mc * 128:(mc + 1) * 128],
                                aug_q[:, cols],
                                start=True, stop=False)
                            nc.tensor.matmul(
                                sc[:, nb * 512:(nb + 1) * 512],
                                idbf16,
                                mask_sb[:, mc * N + qb * QB + nb * 512:
                                        mc * N + qb * QB + (nb + 1) * 512],
                                start=False, stop=True)
                        if mc > 0:
                            pv(mc - 1)
                        pm = ppool.tile([128, QB], BF16,
                                        name=f"pm_{tag}_{qb}_{mc}", tag="pm")
                        nc.scalar.activation(pm, sc, AF.Exp)
                        pms[mc] = pm
                        for f in inject.get((qb, mc), ()):
                            f()
                    pv(NT - 1)
                    out_cb(qb, ot_ps)

            def drain_recip(ot_ps, tag, leaky):
                """PSUM drain (+fused leaky) -> lnum [65,QB] SBUF, plus the
                denominator reciprocal broadcast to [64, QB]."""
                lnum = rowp.tile([65, QB], F32, name=f"ln_{tag}", tag="lnum")
                nc.vector.tensor_copy(lnum, ot_ps)
                # denominators: row -> DRAM -> [128, QB/128] so the reciprocal
                # runs on all 128 lanes (single-lane reciprocal is ~6.5us).
                rd = drb.tile([1, QB], F32, name=f"rd_{tag}", tag="rd")
                nc.sync.dma_start(rd, lnum[64:65, :])
                dn = small.tile([128, QB // 128], F32, name=f"dn_{tag}", tag="dn")
                nc.sync.dma_start(dn, rd.rearrange("o (c p) -> p (o c)", p=128))
                rc = small.tile([128, QB // 128], F32, name=f"rc_{tag}", tag="rc")
                nc.vector.reciprocal(rc, dn)
                rd2 = drb.tile([1, QB], F32, name=f"rd2_{tag}", tag="rd2")
                nc.sync.dma_start(rd2.rearrange("o (c p) -> p (o c)", p=128), rc)
                recb = rowp.tile([64, QB], F32, name=f"rb_{tag}", tag="recb")
                nc.sync.dma_start(recb, rd2.to_broadcast([64, QB]))
                if leaky:
                    # leaky before the (positive) denominator divide is exact
                    nc.vector.scalar_tensor_tensor(
                        lnum[0:64, :], lnum[0:64, :], 0.2, lnum[0:64, :],
                        op0=ALU.mult, op1=ALU.max)
                return lnum, recb

            # ---- layer-1 callback --------------------------------------------
            def make_l1_cb(h):
                def cb(qb, ot_ps):
                    lnum, recb = drain_recip(ot_ps, f"l1_{h}_{qb}",
                                             leaky=not use_bh)
                    dst = zT[h // 2][(h % 2) * 64:(h % 2) * 64 + 64,
                                    qb * QB:(qb + 1) * QB]
                    nc.gpsimd.tensor_tensor(dst, lnum[0:64, :], recb,
                                            op=ALU.mult)
                    if use_bh:
                        nc.vector.tensor_scalar_add(
                            dst, dst, bh_cols[(h % 2) * 64:(h % 2) * 64 + 64,
                                              h // 2:h // 2 + 1])
                        nc.vector.scalar_tensor_tensor(
                            dst, dst, 0.2, dst, op0=ALU.mult, op1=ALU.max)
                return cb

            # ---- layer-2 callback + epilogue ---------------------------------
            def l2_cb(qb, ot_ps):
                lnum, recb = drain_recip(ot_ps, f"l2_{qb}", leaky=not use_bo)
                z2T = rowp.tile([64, QB], F32, name=f"z2T_{qb}", tag="z2T")
                nc.gpsimd.tensor_tensor(z2T, lnum[0:64, :], recb, op=ALU.mult)
                TP = ps_ot.tile([128, (QB // 128) * 64], F32,
                                name=f"TP_{qb}", tag="ot")
                stg = stage.tile([128, (QB // 128) * D], F32,
                                 name=f"stg_{qb}", tag="stg")
                for j in range(QB // 128):
                    nc.tensor.transpose(TP[:, j * 64:(j + 1) * 64],
                                        z2T[:, j * 128:(j + 1) * 128],
                                        idf32[:64, :64])
                    z2p = TP[:, j * 64:(j + 1) * 64]
                    z2 = small.tile([128, D], F32, name=f"z2s_{qb}_{j}",
                                    tag="z2s")
                    s1 = small.tile([128, 1], F32, name=f"s1_{qb}_{j}", tag="s1")
                    if use_bo:
                        nc.vector.tensor_add(z2, z2p, bo_row)
                        nc.vector.scalar_tensor_tensor(
                            z2, z2, 0.2, z2, op0=ALU.mult, op1=ALU.max)
                        nc.vector.tensor_reduce(s1, z2, axis=AX.X, op=ALU.add)
                    else:
                        # SBUF copy + row-sum in one op
                        nc.vector.tensor_scalar(z2, z2p, 1.0, None,
                                                op0=ALU.mult, op1=ALU.add,
                                                accum_out=s1)
                    zsq = small.tile([128, D], F32, name=f"zq_{qb}_{j}", tag="zsq")
                    nc.vector.tensor_mul(zsq, z2, z2)
                    m2 = small.tile([128, 1], F32, name=f"m2_{qb}_{j}", tag="m2")
                    nc.vector.tensor_reduce(m2, zsq, axis=AX.X, op=ALU.add)
                    mu = small.tile([128, 1], F32, name=f"mu_{qb}_{j}", tag="mu")
                    nc.vector.tensor_scalar_mul(mu, s1, 1.0 / D)
                    mq = small.tile([128, 1], F32, name=f"mq_{qb}_{j}", tag="mq")
                    nc.vector.tensor_scalar(mq, mu, mu, -LN_EPS,
                                            op0=ALU.mult, op1=ALU.add)
                    varp = small.tile([128, 1], F32, name=f"vp_{qb}_{j}",
                                      tag="vp")
                    nc.vector.tensor_scalar(varp, m2, 1.0 / D, mq,
                                            op0=ALU.mult, op1=ALU.subtract)
                    # rstd = exp(-0.5 ln(var+eps)) — stays in exp/ln table set
                    lnv = small.tile([128, 1], F32, name=f"lv_{qb}_{j}", tag="lv")
                    nc.scalar.activation(lnv, varp, AF.Ln)
                    rstd = small.tile([128, 1], F32, name=f"rs_{qb}_{j}",
                                      tag="rs")
                    nc.scalar.activation(rstd, lnv, AF.Exp, scale=-0.5)
                    o = stg[:, j * D:(j + 1) * D]
                    nc.vector.tensor_scalar(o, z2, mu, rstd,
                                            op0=ALU.subtract, op1=ALU.mult)
                    if use_gamma:
                        nc.vector.tensor_mul(o, o, gamma_row)
                    if use_beta:
                        nc.vector.tensor_add(o, o, beta_row)
                dst = out_d[qb * QB:(qb + 1) * QB, :].rearrange(
                    "(j p) d -> p j d", p=128)
                nc.sync.dma_start(dst, stg.rearrange("p (j d) -> p j d", j=QB // 128))

            # ---- projection closures -----------------------------------------
            def l1_proj(h):
                def f(p, ps):
                    j0 = p * 512
                    nc.tensor.matmul(ps, whT_sb[:, h * D:(h + 1) * D],
                                     xT[:, j0:j0 + 512], start=True, stop=True)
                return f

            def l2_proj(p, ps):
                j0 = p * 512
                for kc in range(2):
                    nc.tensor.matmul(ps, woT_sb[:, kc * D:(kc + 1) * D],
                                     zT[kc][:, j0:j0 + 512],
                                     start=(kc == 0), stop=(kc == 1))

            # injection schedule: stages s (0..16) of the NEXT aug spread over
            # qb0's mc steps for layer-1 heads.
            L1_SLOTS = [(0, 2), (0, 3), (0, 4), (0, 5), (0, 6), (0, 7),
                        (0, 8), (0, 9), (0, 10), (0, 11), (0, 12), (0, 13),
                        (0, 14), (1, 0), (1, 1), (1, 2), (1, 3)]
            # layer-2 aug: the first-half stages (parts 0/1) go into head 3's
            # qb1 loop; parts 2/3 (which need qb1's zT) run after the pass.
            L2A_STAGES = [0, 1, 3, 2, 5, 8, 13, 14]    # indices into stages
            L2A_SLOTS = [(1, 3), (1, 4), (1, 5), (1, 6), (1, 7), (1, 8),
                         (1, 9), (1, 10)]
            L2B_STAGES = [4, 6, 7, 9, 10, 11, 12, 15, 16]

            # ---- emit ---------------------------------------------------------
            cur = make_aug(l1_proj(0), "l1h0")
            for s in cur[3]:
                s()
            aug2 = None
            for h in range(H):
                inject = {}
                if h < H - 1:
                    nxt = make_aug(l1_proj(h + 1), f"l1h{h + 1}")
                    for i, s in enumerate(nxt[3]):
                        inject.setdefault(L1_SLOTS[i], []).append(s)
                else:
                    aug2 = make_aug(l2_proj, "l2")
                    for si, slot in zip(L2A_STAGES, L2A_SLOTS):
                        inject.setdefault(slot, []).append(aug2[3][si])
                attention(cur[0], cur[1], cur[2], make_l1_cb(h), f"l1h{h}",
                          inject)
                if h < H - 1:
                    cur = nxt
            for si in L2B_STAGES:
                aug2[3][si]()
            attention(aug2[0], aug2[1], aug2[2], l2_cb, "l2")

    return nc


# ---------------------------------------------------------------------------
# Host-side runner (cached compiled executable via bass2jax/PJRT)
# ---------------------------------------------------------------------------
_RUNNER_CACHE = {}


def _make_runner(nc, n_cores):
    import jax
    from jax.sharding import Mesh, PartitionSpec
    from jax.experimental.shard_map import shard_map
    from concourse import bass2jax
    from concourse.bass2jax import _bass_exec_p, install_neuronx_cc_hook

    install_neuronx_cc_hook()
    partition_name = nc.partition_id_tensor.name if nc.partition_id_tensor else None

    in_names, out_names, out_avals = [], [], []
    for alloc in nc.m.functions[0].allocations:
        if not isinstance(alloc, mybir.MemoryLocationSet):
            continue
        name = alloc.memorylocations[0].name
        if alloc.kind == "ExternalInput":
            if name != partition_name:
                in_names.append(name)
        elif alloc.kind == "ExternalOutput":
            out_names.append(name)
            out_avals.append(jax.core.ShapedArray(tuple(alloc.tensor_shape),
                                                  mybir.dt.np(alloc.dtype)))
    n_params = len(in_names)
    n_outs = len(out_avals)
    all_in_names = list(in_names) + list(out_names)
    if partition_name is not None:
        all_in_names.append(partition_name)

    def _body(*args):
        operands = list(args)
        if partition_name is not None:
            operands.append(bass2jax.partition_id_tensor())
        outs = _bass_exec_p.bind(
            *operands,
            out_avals=tuple(out_avals),
            in_names=tuple(all_in_names),
            out_names=tuple(out_names),
            lowering_input_output_aliases=(),
            sim_require_finite=True,
            sim_require_nnan=True,
            nc=nc,
        )
        return tuple(outs)

    donate = tuple(range(n_params, n_params + n_outs))

    if n_cores == 1:
        jitted = jax.jit(_body, donate_argnums=donate, keep_unused=True)

        def run(in_maps):
            args = [np.asarray(in_maps[0][n]) for n in in_names]
            zeros = [np.zeros(a.shape, a.dtype) for a in out_avals]
            outs = jitted(*args, *zeros)
            jax.block_until_ready(outs)
            return [{n: np.asarray(outs[i]) for i, n in enumerate(out_names)}]

        return run

    devices = jax.devices()[:n_cores]
    mesh = Mesh(np.asarray(devices), ("core",))
    in_specs = (PartitionSpec("core"),) * (n_params + n_outs)
    out_specs = (PartitionSpec("core"),) * n_outs
    jitted = jax.jit(
        shard_map(_body, mesh=mesh, in_specs=in_specs, out_specs=out_specs,
                  check_rep=False),
        donate_argnums=donate,
        keep_unused=True,
    )

    def run(in_maps):
        per_core = [[np.asarray(m[n]) for n in in_names] for m in in_maps]
        concat_in = [np.concatenate([per_core[c][i] for c in range(n_cores)], axis=0)
                     for i in range(n_params)]
        concat_zero = [np.zeros((a.shape[0] * n_cores,) + a.shape[1:], a.dtype)
                       for a in out_avals]
        outs = jitted(*concat_in, *concat_zero)
        jax.block_until_ready(outs)
        results = []
        for c in range(n_cores):
            d = {}
            for i, n in enumerate(out_names):
                per_len = out_avals[i].shape[0]
                d[n] = np.asarray(outs[i][c * per_len:(c + 1) * per_len])
            results.append(d)
        return results

    return run


def _get_runner(flags, n_cores):
    key = (flags, n_cores)
    if key not in _RUNNER_CACHE:
        nc = build_gat(use_bh=flags[0], use_bo=flags[1],
                       use_gamma=flags[2], use_beta=flags[3])
        _RUNNER_CACHE[key] = (_make_runner(nc, n_cores), nc)
    return _RUNNER_CACHE[key][0]


def make_in_maps(x, graph, Wh, bh, Wo, bo, gamma, beta):
    B, N, C = x.shape
    H, D, _ = Wh.shape
    flags = (bool(np.any(bh)), bool(np.any(bo)),
             bool(np.any(gamma != 1.0)), bool(np.any(beta)))
    mask = (graph + np.eye(N, dtype=graph.dtype)) > 0
    # additive log-mask: 0 where connected, -60 where masked (folded into
    # the score matmul on the PE; exp(-60+s') underflows to exactly 0)
    maskt = np.ascontiguousarray(
        (mask.T.astype(np.float32) - 1.0) * 60.0).astype(ml_dtypes.bfloat16)
    # whT_sb[c, h*D+d] = Wh[h, d, c]
    wht = np.ascontiguousarray(np.transpose(Wh, (2, 0, 1)).reshape(C, H * D)).astype(np.float32)
    # woT_sb[p, kc*D+d] = Wo[d, kc*128+p]
    wot = np.ascontiguousarray(
        Wo.T.reshape(2, 128, D).transpose(1, 0, 2).reshape(128, 2 * D)).astype(np.float32)
    in_maps = []
    for b in range(B):
        m = {"xt": np.ascontiguousarray(x[b].T).astype(np.float32),
             "maskt": maskt, "wht": wht, "wot": wot}
        if flags[0]:
            m["bh"] = np.ascontiguousarray(
                np.asarray(bh, np.float32).reshape(-1).reshape(2, 128).T)
        if flags[1]:
            m["bo"] = np.asarray(bo, np.float32)
        if flags[2]:
            m["gamma"] = np.asarray(gamma, np.float32)
        if flags[3]:
            m["beta"] = np.asarray(beta, np.float32)
        in_maps.append(m)
    return in_maps, flags


def kernel(x, graph, Wh, bh, Wo, bo, gamma, beta):
    x = np.asarray(x)
    B = x.shape[0]
    in_maps, flags = make_in_maps(np.asarray(x, np.float32), np.asarray(graph),
                                  np.asarray(Wh, np.float32),
                                  np.asarray(bh, np.float32),
                                  np.asarray(Wo, np.float32),
                                  np.asarray(bo, np.float32),
                                  np.asarray(gamma, np.float32),
                                  np.asarray(beta, np.float32))
    run = _get_runner(flags, B)
    results = run(in_maps)
    return np.stack([r["out"] for r in results], axis=0)

